# revision 18
# baseline (speedup 1.0000x reference)
"""Causal single-head attention (n=8192, d_model=1024, d_head=128) on 8 TRN2 cores.

Strategy (sequence-parallel, SPMD-uniform):
  - Core c owns query rows {8i + c : i in [0, 1024)} (mod-8 interleave).
    Causal work per row ~ q/128 tiles, so interleaving balances cores exactly
    and every core runs the *identical* instruction stream; per-core
    differences enter only through data (gathered xTq columns + mask tiles).
  - All tensors kept "transposed" so no on-chip transposes are needed in the
    attention loop:
       Q^T[h, i]   = W_q^T x^T        (lhsT = W_q k-tile,   rhs = xT chunk)
       K^T[h, j]   = W_k^T x^T        (same form, all 8192 keys, replicated)
       V[j, h]     = transpose(V^T)   (64 PE transposes, once)
       S^T[j, q]   = (K^T_J)^T Q^T    (lhsT = K^T 128-tile, rhs = Q^T cols)
       expS        = exp(S^T / sqrt(d))    (ACT, reads PSUM, writes SBUF)
       O^T[h, q]  += V_J^T expS       (lhsT = V j-tile,     rhs = expS)
       Z[q]       += ones^T expS      (lhsT = ones[128,1],  rhs = expS)
       y[q, d]     = (O^T/Z)^T W_o    (normalize O^T by 1/Z, then project)
  - No softmax max-subtraction: scores are ~N(0,1) (|s| < ~8), exp is safe.
  - Causal masking: only j-tiles with j <= q are computed; the single
    diagonal tile per (J, q-half) is masked multiplicatively after exp.
    128-wide tail blocks are widened to 256 (fp32r needs >=256 moving cols
    for full PE rate) with the extra, fully-masked 128 columns zeroed.
  - float32r-typed matmul operands (fp32 bits, FP22 in PE, fp32 PSUM accum).
  - Inputs are host pre-tiled so every big DMA is contiguous per partition.
"""

import numpy as np

N_CTX = 8192
D_MODEL = 1024
D_HEAD = 128
NCORES = 8
P = 128
KT = D_MODEL // P          # 8 k-tiles of the contraction dim
R = N_CTX // NCORES        # 1024 query rows per core
RCH = 512                  # x^T column chunk (rows of x) per projection step
NCHUNK = N_CTX // RCH      # 16
NJ = N_CTX // P            # 64 key tiles
HALF = R // 2              # 512 query columns per attention pass
INV_SQRT_D = float(1.0 / np.sqrt(D_HEAD))

# att2 backfill (J' < 32) distribution over chunks n=8..15, balancing PE work
_BACKFILL = [2, 3, 3, 4, 4, 5, 5, 6]
_BF_STARTS = np.concatenate([[0], np.cumsum(_BACKFILL)]).tolist()

_CACHE = {}


def _build(loop_iters=None, parts=frozenset({'q','k','v','s','pv','z','out'})):
    """Build + compile the SPMD program (one program, 8 cores).

    loop_iters: if set, wrap the whole body in a hardware loop that runs it
    that many times back-to-back — used only for steady-state HW timing.
    """
    import contextlib
    from contextlib import ExitStack

    import concourse.bass as bass
    import concourse.mybir as mybir
    import concourse.tile as tile
    from concourse import bacc

    f32 = mybir.dt.float32
    f32r = mybir.dt.float32r
    Exp = mybir.ActivationFunctionType.Exp

    nc = bacc.Bacc("TRN2", target_bir_lowering=False, debug=False,
                   num_devices=NCORES)

    # host pre-tiled inputs (see _host_in_maps); matmul operands are f32r
    xt = nc.dram_tensor("xt", [NCHUNK, P, KT, RCH], f32r, kind="ExternalInput")
    xq = nc.dram_tensor("xq", [2, KT, P, RCH], f32r, kind="ExternalInput")
    wq = nc.dram_tensor("wq", [P, KT, P], f32r, kind="ExternalInput")
    wk = nc.dram_tensor("wk", [P, KT, P], f32r, kind="ExternalInput")
    wv = nc.dram_tensor("wv", [P, KT, P], f32r, kind="ExternalInput")
    wo = nc.dram_tensor("wo", [D_HEAD, D_MODEL], f32r, kind="ExternalInput")
    masks = nc.dram_tensor("masks", [P, 9, P], f32, kind="ExternalInput")
    eye = nc.dram_tensor("eye", [P, P], f32r, kind="ExternalInput")
    ones = nc.dram_tensor("ones", [P, 1], f32r, kind="ExternalInput")
    y = nc.dram_tensor("y", [R, D_MODEL], f32, kind="ExternalOutput")

    with tile.TileContext(nc) as tc, ExitStack() as ctx:
        consts = ctx.enter_context(tc.tile_pool(name="consts", bufs=1))
        xpool = ctx.enter_context(tc.tile_pool(name="xpool", bufs=3))
        vpool = ctx.enter_context(tc.tile_pool(name="vpool", bufs=3))
        sepool = ctx.enter_context(tc.tile_pool(name="sepool", bufs=4))
        ppj = ctx.enter_context(tc.tile_pool(name="ppj", bufs=2, space="PSUM"))
        pss = ctx.enter_context(tc.tile_pool(name="pss", bufs=2, space="PSUM"))
        pacc = ctx.enter_context(tc.tile_pool(name="pacc", bufs=1, space="PSUM"))

        # ---- persistent SBUF ----
        wq_sb = consts.tile([P, KT, P], f32r, tag="wq")
        wk_sb = consts.tile([P, KT, P], f32r, tag="wk")
        wv_sb = consts.tile([P, KT, P], f32r, tag="wv")
        wo_sb = consts.tile([P, D_MODEL], f32r, tag="wo")
        masks_sb = consts.tile([P, 9, P], f32, tag="masks")
        eye_sb = consts.tile([P, P], f32r, tag="eye")
        ones_sb = consts.tile([P, 1], f32r, tag="ones")
        ones1_sb = consts.tile([1, P], f32, tag="ones1")
        qT_sb = consts.tile([P, R], f32r, tag="qT")
        kT_sb = consts.tile([P, N_CTX], f32r, tag="kT")
        v_sb = consts.tile([P, NJ, P], f32r, tag="v")
        oT_sb = consts.tile([P, R], f32, tag="oT")
        oTn_sb = consts.tile([P, R], f32r, tag="oTn")
        zr_sb = consts.tile([1, R], f32, tag="zr")

        # PSUM accumulators: O^T and Z, one half live at a time (shared slots)
        acc = {}

        def get_acc(hf):
            if hf not in acc:
                acc[hf] = (pacc.tile([P, HALF], f32, tag="oT",
                                     name=f"oT_ps{hf}"),
                           pacc.tile([1, HALF], f32, tag="z",
                                     name=f"z_ps{hf}"))
            return acc[hf]

        def load_q_chunk(m):
            """Q^T columns [512m, 512m+512) from host-gathered xq, per-ktile DMAs."""
            xq_t = xpool.tile([P, KT, RCH], f32r, tag="xt", name="xq_t")
            for kt in range(KT):
                nc.sync.dma_start(out=xq_t[:, kt, :], in_=xq[m, kt])
            q_ps = ppj.tile([P, RCH], f32, tag="pj", name="q_ps")
            for kt in range(KT):
                nc.tensor.matmul(q_ps, wq_sb[:, kt, :], xq_t[:, kt, :],
                                 start=(kt == 0), stop=(kt == KT - 1))
            nc.vector.tensor_copy(qT_sb[:, m * RCH:(m + 1) * RCH], q_ps)

        def attention_group(hf, Js):
            """A uniform group of key-tiles Js against query-half hf.

            All Js must share the same query-column window (same start_t).
            Structure: S matmuls into paired 2-bank PSUM tiles, one exp per
            pair, masks, then batched PV and Z matmuls (Z shares one
            stationary ones-vector load across the group).
            """
            t0 = Js[0] // 8
            start_t = max(t0, 4 * hf)
            widened = (start_t == 4 * hf + 3)  # 128-wide tail: widen to 256
            if widened:
                start_t -= 1
            col0 = P * start_t              # global qT column
            w = HALF * (hf + 1) - col0      # 512/384/256(widened)
            lc0 = col0 - HALF * hf          # column inside this half's accums
            assert all(max(J // 8, 4 * hf) == max(t0, 4 * hf) for J in Js)
            off1 = RCH if w > 256 else w    # second-in-pair region offset
            diag_grp = (t0 >= 4 * hf)
            regions = []                    # (J, se_tile, offset)
            for pi in range(0, len(Js), 2):
                pair = Js[pi:pi + 2]
                se = sepool.tile([P, 2 * RCH], f32r, tag="se", name="se")
                if 's' in parts:
                    s_ps = pss.tile([P, 2 * RCH], f32, tag="s", name="s_ps")
                    for idx, J in enumerate(pair):
                        off = idx * off1
                        nc.tensor.matmul(s_ps[:, off:off + w],
                                         kT_sb[:, J * P:(J + 1) * P],
                                         qT_sb[:, col0:col0 + w],
                                         start=True, stop=True)
                    if off1 == w or len(pair) == 1:   # contiguous regions
                        width = off1 * (len(pair) - 1) + w
                        nc.scalar.activation(se[:, :width], s_ps[:, :width],
                                             Exp, scale=INV_SQRT_D)
                    else:                             # gapped (w=384): split
                        for idx in range(len(pair)):
                            off = idx * off1
                            nc.scalar.activation(se[:, off:off + w],
                                                 s_ps[:, off:off + w],
                                                 Exp, scale=INV_SQRT_D)
                    for idx, J in enumerate(pair):
                        off = idx * off1
                        if widened:     # zero padding tile + diagonal mask
                            nc.vector.tensor_mul(se[:, off:off + P],
                                                 se[:, off:off + P],
                                                 masks_sb[:, 8, :])
                            nc.vector.tensor_mul(se[:, off + P:off + 2 * P],
                                                 se[:, off + P:off + 2 * P],
                                                 masks_sb[:, J % 8, :])
                        elif diag_grp:  # diagonal mask on first 128 cols
                            nc.vector.tensor_mul(se[:, off:off + P],
                                                 se[:, off:off + P],
                                                 masks_sb[:, J % 8, :])
                for idx, J in enumerate(pair):
                    regions.append((J, se, idx * off1))
            oT_a, z_a = get_acc(hf)
            last_J = 32 * (hf + 1) - 1
            if 'pv' in parts:
                for (J, se, off) in regions:
                    nc.tensor.matmul(oT_a[:, lc0:lc0 + w], v_sb[:, J, :],
                                     se[:, off:off + w],
                                     start=(J == 0), stop=(J == last_J))
            if 'z' in parts:
                # sum the group's se tiles on DVE, then one ones-matmul
                if len(regions) == 1:
                    _, se0, off0 = regions[0]
                    zrhs = se0[:, off0:off0 + w]
                else:
                    zs = vpool.tile([P, RCH], f32r, tag="zsum", name="zsum")
                    _, se0, off0 = regions[0]
                    _, se1, off1_ = regions[1]
                    nc.vector.tensor_add(zs[:, :w], se0[:, off0:off0 + w],
                                         se1[:, off1_:off1_ + w])
                    for (J, se_i, off_i) in regions[2:]:
                        nc.vector.tensor_add(zs[:, :w], zs[:, :w],
                                             se_i[:, off_i:off_i + w])
                    zrhs = zs[:, :w]
                nc.tensor.matmul(z_a[0:1, lc0:lc0 + w], ones_sb[:, 0:1], zrhs,
                                 start=(0 in Js), stop=(last_J in Js))

        def normalize(hf):
            """Normalize accumulated O^T half by 1/Z into oTn_sb."""
            c0 = hf * HALF
            oT_a, z_a = get_acc(hf)
            nc.vector.tensor_copy(oT_sb[:, c0:c0 + HALF], oT_a)
            nc.vector.reciprocal(zr_sb[0:1, c0:c0 + HALF], z_a[0:1, :])
            zb_ps = ppj.tile([P, HALF], f32, tag="pj", name="zb_ps")
            nc.tensor.matmul(zb_ps, ones1_sb[0:1, :],
                             zr_sb[0:1, c0:c0 + HALF], start=True, stop=True)
            nc.vector.tensor_mul(oTn_sb[:, c0:c0 + HALF],
                                 oT_sb[:, c0:c0 + HALF], zb_ps)

        def y_project(hf, i):
            """i-th (query-tile, d-chunk) output block of half hf."""
            qt = 4 * hf + i // 2
            dc = i % 2
            tg = "s" if i % 2 == 0 else "pj"
            pool = pss if tg == "s" else ppj
            y_ps = pool.tile([P, RCH], f32, tag=tg, name="y_ps")
            nc.tensor.matmul(y_ps, oTn_sb[:, qt * P:(qt + 1) * P],
                             wo_sb[:, dc * RCH:(dc + 1) * RCH],
                             start=True, stop=True)
            y_sb = vpool.tile([P, RCH], f32, tag="vt", name="y_sb")
            nc.scalar.copy(y_sb, y_ps)
            nc.sync.dma_start(out=y[qt * P:(qt + 1) * P,
                                    dc * RCH:(dc + 1) * RCH], in_=y_sb)

        def load_xt(n):
            xt_t = xpool.tile([P, KT, RCH], f32r, tag="xt", name="xt_t")
            nc.sync.dma_start(out=xt_t.rearrange("p kt r -> p (kt r)"),
                              in_=xt[n].rearrange("p kt r -> p (kt r)"))
            return xt_t

        def body():
            acc.clear()           # fresh accumulator tiles per (timing) pass
            pending = load_xt(0)  # chunk-0 x^T ahead of everything else
            if 'q' in parts:
                load_q_chunk(0)   # attention half 1 needs only Q^T[:, :512]
            nc.sync.dma_start(out=masks_sb, in_=masks[:, :, :])
            nc.sync.dma_start(out=eye_sb, in_=eye[:, :])
            nc.sync.dma_start(out=ones_sb, in_=ones[:, :])
            nc.sync.dma_start(out=wo_sb, in_=wo[:, :])
            for n in range(NCHUNK):
                xt_t = pending
                if n + 1 < NCHUNK:
                    pending = load_xt(n + 1)
                if 'k' in parts:
                    kt_ps = ppj.tile([P, RCH], f32, tag="pj", name="kt_ps")
                    for kt in range(KT):
                        nc.tensor.matmul(kt_ps, wk_sb[:, kt, :], xt_t[:, kt, :],
                                         start=(kt == 0), stop=(kt == KT - 1))
                    nc.vector.tensor_copy(kT_sb[:, n * RCH:(n + 1) * RCH], kt_ps)
                if 'v' in parts:
                    vt_ps = ppj.tile([P, RCH], f32, tag="pj", name="vt_ps")
                    for kt in range(KT):
                        nc.tensor.matmul(vt_ps, wv_sb[:, kt, :], xt_t[:, kt, :],
                                         start=(kt == 0), stop=(kt == KT - 1))
                    vt_tmp = vpool.tile([P, RCH], f32r, tag="vt", name="vt_tmp")
                    nc.vector.tensor_copy(vt_tmp, vt_ps)
                    for t in range(RCH // P):  # V^T -> V transpose
                        vtr_ps = ppj.tile([P, P], f32r, tag="pj", name="vtr_ps")
                        nc.tensor.transpose(vtr_ps, vt_tmp[:, t * P:(t + 1) * P],
                                            eye_sb)
                        nc.vector.tensor_copy(v_sb[:, 4 * n + t, :], vtr_ps)

                if n == 1:
                    load_q_chunk(1)   # Q^T[:, 512:] needed from chunk 8 on
                if n < 8:
                    attention_group(0, list(range(4 * n, 4 * n + 4)))
                else:
                    i = n - 8
                    bf = list(range(_BF_STARTS[i], _BF_STARTS[i + 1]))
                    if bf:
                        attention_group(1, bf)   # backfill J<32 for half 1
                    attention_group(1, list(range(4 * n, 4 * n + 4)))
                if 'out' in parts:
                    if n == 7:
                        normalize(0)
                    elif n >= 8:
                        y_project(0, n - 8)   # spread half-1 output
            if 'out' in parts:
                normalize(1)
                for i in range(8):
                    y_project(1, i)

        # ---- emission: DMA-priority-aware ordering ----
        nc.sync.dma_start(out=wq_sb, in_=wq[:, :, :])
        nc.sync.dma_start(out=wk_sb, in_=wk[:, :, :])
        nc.sync.dma_start(out=wv_sb, in_=wv[:, :, :])
        nc.vector.memset(ones1_sb, 1.0)
        if loop_iters:
            with tc.For_i(0, loop_iters, 1):
                body()
        else:
            body()

    nc.compile()
    return nc


def _get_nc():
    if "nc" not in _CACHE:
        _CACHE["nc"] = _build()
    return _CACHE["nc"]


def _host_in_maps(x, W_q, W_k, W_v, W_o):
    x = np.asarray(x, dtype=np.float32)
    xT = np.ascontiguousarray(x.T)                       # [1024, 8192]
    # [n, p, kt, r] = xT[kt*128+p, n*512+r]
    xt = np.ascontiguousarray(
        xT.reshape(KT, P, NCHUNK, RCH).transpose(2, 1, 0, 3))
    def wtile(w):
        return np.ascontiguousarray(
            np.asarray(w, np.float32).reshape(KT, P, D_HEAD).transpose(1, 0, 2))
    wq_t, wk_t, wv_t = wtile(W_q), wtile(W_k), wtile(W_v)
    W_o = np.ascontiguousarray(np.asarray(W_o, dtype=np.float32))
    eye = np.eye(P, dtype=np.float32)
    pp = np.arange(P)[:, None, None]
    uu = np.arange(8)[None, :, None]
    di = np.arange(P)[None, None, :]
    in_maps = []
    for c in range(NCORES):
        xTq = xT[:, c::NCORES]                           # [1024, 1024]
        # [m, kt, p, r] = xTq[kt*128+p, m*512+r]
        xq = np.ascontiguousarray(
            xTq.reshape(KT, P, 2, RCH).transpose(2, 0, 1, 3))
        mask_c = (8 * di + c >= 128 * uu + pp).astype(np.float32)  # [p, u, di]
        mask_c = np.concatenate([mask_c, np.zeros((P, 1, P), np.float32)], axis=1)
        in_maps.append({
            "xt": xt, "xq": xq,
            "wq": wq_t, "wk": wk_t, "wv": wv_t, "wo": W_o,
            "masks": np.ascontiguousarray(mask_c),
            "eye": eye,
            "ones": np.ones((P, 1), np.float32),
        })
    return in_maps


def _run(x, W_q, W_k, W_v, W_o, trace=False):
    from concourse.bass_utils import run_bass_kernel_spmd
    nc = _get_nc()
    in_maps = _host_in_maps(x, W_q, W_k, W_v, W_o)
    res = run_bass_kernel_spmd(nc, in_maps, list(range(NCORES)), trace=trace)
    out = np.empty((N_CTX, D_MODEL), dtype=np.float32)
    for c in range(NCORES):
        out[c::NCORES] = res.results[c]["y"]
    return out, res


def kernel(x, W_q, W_k, W_v, W_o):
    out, _ = _run(x, W_q, W_k, W_v, W_o, trace=False)
    return out
